# revision 16
# baseline (speedup 1.0000x reference)
"""MinimalRNNCell on 8 Trainium2 NeuronCores.

h_t = x_t @ W + h_{t-1} @ R, h_0 = 0, for x: [B=32, T=1024, D=512],
W: [D, U=512], R: [U, U]. Returns all h_t -> [B, T, U] float32.

Strategy (data-parallel over batch, chunked linear scan over time):
  - Shard batch over 8 cores (4 rows each). All matmul work runs in the
    transposed layout h^T[U, r] so R/W stay natural as the stationary
    operand and nothing is transposed on device. The host pre-permutes
    x into xr[c, d, r] with r = (chunk j, batch b), t = j*C + c, so
    every DMA is contiguous.
  - Phase A: C sequential steps; step c advances all L chunks at once:
    hloc_c = x_c @ W + hloc_{c-1} @ R as one PSUM accumulation group
    per 128-row output block (fat [512]-wide moving operands).
  - Phase B: chunk-boundary carry via a Kogge-Stone doubling scan with
    host-precomputed powers R^(C*2^k). R is strongly contractive here,
    so powers below a tolerance are dropped — typically only ~3 rounds
    survive, each a fat batched matmul (no thin sequential carry).
  - Phase C: C steps of corrections G_c = G_{c-1} @ R seeded with the
    carry states; h_c = hloc_c + G_c is fused into the PSUM drain and
    streamed straight out to DRAM.

Matmul dtype is selectable via RNN_MM_DTYPE: "f32" (exact, 4 cyc/row),
"f32r" (TF32 mode, full rate at N>=256), "bf16".
"""

import os

import numpy as np

import concourse.bass as bass
import concourse.mybir as mybir
import concourse.tile as tile
from concourse import bass_utils

B, T, D, U = 32, 1024, 512, 512
NCORES = 8
BLOC = B // NCORES  # 4 batch rows per core
C = 8  # intra-chunk steps (phase A/C length)
L = T // C  # 128 chunks
RCOLS = BLOC * L  # 512 moving columns
NCH = U // 128  # 4 partition chunks of the 512-dim
POW_TOL = 1e-4  # drop Kogge-Stone rounds with ||R^(C*2^k)||_2 below this
MAX_SYNC_WAITS = 1

MM_DTYPE = os.environ.get("RNN_MM_DTYPE", "f32r")
# debug: which phases to build ("aw" = phase A without recurrence MMs,
# "a", "ab", "abc" = full kernel)
PHASES = os.environ.get("RNN_PHASES", "abc")


def _split_sync_waits(nc, max_waits=MAX_SYNC_WAITS):
    """Walrus rejects instructions carrying more than a couple of sync
    waits (CTRL structs in this toolchain cap out below what Tile's
    final drain needs). Hoist excess waits onto single-wait NoOps
    placed immediately before the offending instruction."""
    for fn in nc.m.functions:
        for bb in fn.blocks:
            insts = bb.instructions
            out, changed = [], False
            for inst in insts:
                si = inst.sync_info
                waits = list(si.on_wait) if si is not None else []
                if len(waits) > max_waits:
                    for k, w in enumerate(waits[:-max_waits]):
                        out.append(
                            mybir.InstNoOp(
                                name=f"I-wsplit-{inst.name}-{k}",
                                engine=inst.engine,
                                ins=[],
                                outs=[],
                                sync_info=mybir.SyncInfo(on_wait=[w], on_update=[]),
                            )
                        )
                    inst.sync_info = mybir.SyncInfo(
                        on_wait=waits[-max_waits:], on_update=list(si.on_update)
                    )
                    changed = True
                out.append(inst)
            if changed:
                insts[:] = out


def _build_nc(npow, reps=1):
    f32 = mybir.dt.float32
    if MM_DTYPE == "bf16":
        io_dt = mybir.dt.bfloat16
    elif MM_DTYPE == "f32r":
        io_dt = mybir.dt.float32r
    else:
        io_dt = f32

    def vin(ap):
        # DVE/ACT read of an f32r tile: same bits as f32
        return ap.bitcast(f32) if MM_DTYPE == "f32r" else ap

    nc = bass.Bass("TRN2", target_bir_lowering=False, debug=False)
    xr_d = nc.dram_tensor("xr", [C, D, RCOLS], io_dt, kind="ExternalInput").ap()
    w_d = nc.dram_tensor("w", [D, U], io_dt, kind="ExternalInput").ap()
    r_d = nc.dram_tensor("r", [U, U], io_dt, kind="ExternalInput").ap()
    if npow:
        pw_d = nc.dram_tensor("pows", [npow, U, U], io_dt, kind="ExternalInput").ap()
    hr_d = nc.dram_tensor("hr", [C, U, RCOLS], f32, kind="ExternalOutput").ap()

    # zero-pad in front of the chunk axis so shifted reads in phases B/C
    # fall into zeros instead of needing edge cases (and keep N=RCOLS,
    # which f32r wants >= 256 for full rate)
    pad = BLOC * (1 << max(npow - 1, 0)) if npow else BLOC
    pad = max(pad, BLOC)

    with tile.TileContext(nc) as tc:
      for _rep in range(reps):
        with (
            tc.tile_pool(name=f"wts{_rep}", bufs=2 * 16 + npow * 16) as wpool,
            tc.tile_pool(name=f"hl{_rep}", bufs=C * NCH) as hlpool,
            tc.tile_pool(name=f"xt{_rep}", bufs=2 * NCH) as xtpool,
            tc.tile_pool(name=f"hp{_rep}", bufs=2 * NCH) as hppool,
            tc.tile_pool(name=f"g{_rep}", bufs=2 * NCH) as gpool,
            tc.tile_pool(name=f"out{_rep}", bufs=2 * NCH) as outpool,
            tc.tile_pool(name=f"ps{_rep}", bufs=8, space="PSUM") as pspool,
        ):
            # --- resident weights: one wide DMA per 128-row band, with
            # column-sliced views as the stationary 128x128 blocks; loads
            # are emitted in consumption order so PE starts early ---
            def load_bands(src, name, tag=None, bufs=1):
                views = [[None] * NCH for _ in range(NCH)]
                for a in range(NCH):
                    t = wpool.tile(
                        [128, U], io_dt, tag=tag or f"{name}{a}", bufs=bufs
                    )
                    nc.scalar.dma_start(out=t[:], in_=src[128 * a : 128 * (a + 1), :])
                    for bidx in range(NCH):
                        views[a][bidx] = t[:, 128 * bidx : 128 * (bidx + 1)]
                return views

            def load_x(c):
                xts = []
                for d in range(NCH):
                    t = xtpool.tile([128, RCOLS], io_dt, tag=f"x{d}", bufs=3)
                    nc.sync.dma_start(out=t[:], in_=xr_d[c, 128 * d : 128 * (d + 1), :])
                    xts.append(t)
                return xts

            w_t = load_bands(w_d, "w")
            xpre = {0: load_x(0), 1: load_x(1)}
            r_t = load_bands(r_d, "r")
            pw_t = []

            # --- phase A: intra-chunk local scan ---
            hl = [[None] * NCH for _ in range(C)]
            for c in range(C):
                xts = xpre.pop(c) if c in xpre else load_x(c)
                if c == 2:
                    pw_t.extend(
                        load_bands(pw_d[k], f"p{k}", tag="pw", bufs=8)
                        for k in range(npow)
                    )
                for u in range(NCH):
                    ops = [(w_t[d][u], xts[d]) for d in range(NCH)]
                    if c > 0 and PHASES != "aw":
                        ops += [(r_t[v][u], hl[c - 1][v]) for v in range(NCH)]
                    ps = pspool.tile([128, RCOLS], f32, tag="ps")
                    for i, (lhsT, rhs) in enumerate(ops):
                        nc.tensor.matmul(
                            ps[:], lhsT[:], rhs[:],
                            start=(i == 0), stop=(i == len(ops) - 1),
                        )
                    ht = hlpool.tile([128, RCOLS], io_dt, tag=f"hl{c}_{u}", bufs=1)
                    if (c * NCH + u) % 2 == 0:
                        nc.vector.tensor_copy(out=ht[:], in_=ps[:])
                    else:
                        nc.scalar.copy(out=ht[:], in_=ps[:])
                    hl[c][u] = ht

            if PHASES in ("aw", "a"):
                # debug build: dump hloc as the output, skip B/C
                for c in range(C):
                    for u in range(NCH):
                        ot = outpool.tile([128, RCOLS], f32, tag=f"o{u}", bufs=2)
                        nc.vector.tensor_copy(out=ot[:], in_=vin(hl[c][u][:]))
                        nc.sync.dma_start(
                            out=hr_d[c, 128 * u : 128 * (u + 1), :], in_=ot[:]
                        )
            else:
                # --- phase B: Kogge-Stone carry over chunk ends ---
                hpa, hpb = [], []
                for v in range(NCH):
                    ta = hppool.tile([128, pad + RCOLS], io_dt, tag=f"hpa{v}", bufs=1)
                    tb = hppool.tile([128, pad + RCOLS], io_dt, tag=f"hpb{v}", bufs=1)
                    nc.gpsimd.memset(vin(ta[:, 0:pad]), 0.0)
                    nc.gpsimd.memset(vin(tb[:, 0:pad]), 0.0)
                    nc.vector.tensor_copy(
                        out=ta[:, pad : pad + RCOLS], in_=vin(hl[C - 1][v][:])
                    )
                    hpa.append(ta)
                    hpb.append(tb)
                src, dst = hpa, hpb
                for k in range(npow if PHASES != "ab0" else 0):
                    sh = BLOC * (1 << k)
                    for u in range(NCH):
                        ps = pspool.tile([128, RCOLS], f32, tag="ps")
                        for v in range(NCH):
                            nc.tensor.matmul(
                                ps[:], pw_t[k][v][u][:],
                                src[v][:, pad - sh : pad - sh + RCOLS],
                                start=(v == 0), stop=(v == NCH - 1),
                            )
                        nc.vector.tensor_add(
                            out=dst[u][:, pad : pad + RCOLS], in0=ps[:],
                            in1=vin(src[u][:, pad : pad + RCOLS]),
                        )
                    src, dst = dst, src

                if PHASES == "ab":
                    for c in range(C):
                        for u in range(NCH):
                            ot = outpool.tile([128, RCOLS], f32, tag=f"o{u}", bufs=2)
                            nc.vector.tensor_copy(out=ot[:], in_=vin(hl[c][u][:]))
                            nc.sync.dma_start(
                                out=hr_d[c, 128 * u : 128 * (u + 1), :], in_=ot[:]
                            )
                else:
                    # --- phase C: apply carries, emit h ---
                    prev = [
                        src[v][:, pad - BLOC : pad - BLOC + RCOLS] for v in range(NCH)
                    ]
                    for c in range(C):
                        nxt = []
                        for u in range(NCH):
                            ps = pspool.tile([128, RCOLS], f32, tag="ps")
                            for v in range(NCH):
                                nc.tensor.matmul(
                                    ps[:], r_t[v][u][:], prev[v],
                                    start=(v == 0), stop=(v == NCH - 1),
                                )
                            if c < C - 1:
                                gt = gpool.tile([128, RCOLS], io_dt, tag=f"g{u}", bufs=2)
                                nc.scalar.copy(out=gt[:], in_=ps[:])
                                nxt.append(gt[:])
                            ot = outpool.tile([128, RCOLS], f32, tag=f"o{u}", bufs=2)
                            nc.vector.tensor_add(
                                out=ot[:], in0=ps[:], in1=vin(hl[c][u][:])
                            )
                            nc.sync.dma_start(
                                out=hr_d[c, 128 * u : 128 * (u + 1), :], in_=ot[:]
                            )
                        prev = nxt

    _split_sync_waits(nc)
    return nc


_CACHE = {}


def _get_nc(npow, reps=1):
    key = (npow, MM_DTYPE, PHASES, reps)
    if key not in _CACHE:
        _CACHE[key] = _build_nc(npow, reps)
    return _CACHE[key]


def _tf32_round(a):
    b = np.ascontiguousarray(a, np.float32).view(np.uint32)
    r = ((b >> np.uint32(13)) & np.uint32(1)) + np.uint32(0x0FFF)
    b = (b + r) & np.uint32(0xFFFFE000)
    return b.view(np.float32)


def _cast_host(a):
    if MM_DTYPE == "bf16":
        import ml_dtypes

        return np.ascontiguousarray(a.astype(ml_dtypes.bfloat16))
    if MM_DTYPE == "f32r":
        return np.ascontiguousarray(_tf32_round(a))
    return np.ascontiguousarray(a.astype(np.float32))


def prepare_inputs(x, kernel, recurrent_kernel):
    """Host-side shard + permute. Returns (in_maps, npow)."""
    x = np.asarray(x)
    kernel = np.asarray(kernel)
    recurrent_kernel = np.asarray(recurrent_kernel)
    # Kogge-Stone power ladder R^(C*2^k), computed in fp64; drop
    # negligible rounds (R is contractive so high powers underflow).
    pows = []
    m = np.linalg.matrix_power(recurrent_kernel.astype(np.float64), C)
    for _ in range(L.bit_length() - 1):  # shifts 2^k < L
        if np.linalg.norm(m, 2) <= POW_TOL:
            break
        pows.append(m.astype(np.float32))
        m = m @ m
    npow = len(pows)
    pw = _cast_host(np.stack(pows)) if npow else None
    w = _cast_host(kernel)
    r = _cast_host(recurrent_kernel)
    in_maps = []
    for k in range(NCORES):
        xc = x[BLOC * k : BLOC * (k + 1)]  # [BLOC, T, D]
        # xr[c, d, j*BLOC + b] = xc[b, j*C + c, d]
        xr = _cast_host(
            xc.reshape(BLOC, L, C, D).transpose(2, 3, 1, 0).reshape(C, D, RCOLS)
        )
        im = {"xr": xr, "w": w, "r": r}
        if npow:
            im["pows"] = pw
        in_maps.append(im)
    return in_maps, npow


def assemble_output(results):
    out = np.empty((B, T, U), np.float32)
    for k in range(NCORES):
        hr = results[k]["hr"]  # [C, U, RCOLS]
        # out[b, j*C + c, u] = hr[c, u, j*BLOC + b]
        out[BLOC * k : BLOC * (k + 1)] = (
            hr.reshape(C, U, L, BLOC).transpose(3, 2, 0, 1).reshape(BLOC, T, U)
        )
    return out


_RUNNERS = {}


def _get_runner(nc):
    """Build (once) a sharded jitted executable for `nc` on 8 cores.
    Mirrors bass2jax.run_bass_via_pjrt's multi-core path, but cached so
    repeated kernel() calls don't re-trace/re-compile."""
    if nc in _RUNNERS:
        return _RUNNERS[nc]
    import jax
    from jax.sharding import Mesh, PartitionSpec
    from jax.experimental.shard_map import shard_map
    from concourse import bass2jax

    bass2jax.install_neuronx_cc_hook()
    partition_name = nc.partition_id_tensor.name if nc.partition_id_tensor else None
    in_names, out_names, out_avals = [], [], []
    for alloc in nc.m.functions[0].allocations:
        if not isinstance(alloc, mybir.MemoryLocationSet):
            continue
        name = alloc.memorylocations[0].name
        if alloc.kind == "ExternalInput":
            if name != partition_name:
                in_names.append(name)
        elif alloc.kind == "ExternalOutput":
            out_names.append(name)
            out_avals.append(
                jax.core.ShapedArray(
                    tuple(alloc.tensor_shape), mybir.dt.np(alloc.dtype)
                )
            )
    n_params = len(in_names)
    in_names_all = list(in_names) + out_names
    if partition_name is not None:
        in_names_all.append(partition_name)

    def _body(*args):
        operands = list(args)
        if partition_name is not None:
            operands.append(bass2jax.partition_id_tensor())
        return tuple(
            bass2jax._bass_exec_p.bind(
                *operands,
                out_avals=tuple(out_avals),
                in_names=tuple(in_names_all),
                out_names=tuple(out_names),
                lowering_input_output_aliases=(),
                sim_require_finite=True,
                sim_require_nnan=True,
                nc=nc,
            )
        )

    devices = jax.devices()[:NCORES]
    mesh = Mesh(np.asarray(devices), ("core",))
    nouts = len(out_names)
    sharded = jax.jit(
        shard_map(
            _body,
            mesh=mesh,
            in_specs=(PartitionSpec("core"),) * (n_params + nouts),
            out_specs=(PartitionSpec("core"),) * nouts,
            check_rep=False,
        ),
        keep_unused=True,
    )

    def run(in_maps):
        concat_in = [
            np.concatenate([np.asarray(in_maps[c][nm]) for c in range(NCORES)], axis=0)
            for nm in in_names
        ]
        concat_zero = [
            np.zeros((NCORES * a.shape[0], *a.shape[1:]), a.dtype) for a in out_avals
        ]
        outs = sharded(*concat_in, *concat_zero)
        return [
            {
                nm: np.asarray(outs[i]).reshape(NCORES, *out_avals[i].shape)[c]
                for i, nm in enumerate(out_names)
            }
            for c in range(NCORES)
        ]

    run.sharded = sharded
    run.in_names = list(in_names)
    run.out_shapes = [(tuple(a.shape), a.dtype) for a in out_avals]
    _RUNNERS[nc] = run
    return run


def kernel(x, kernel, recurrent_kernel):
    in_maps, npow = prepare_inputs(x, kernel, recurrent_kernel)
    nc = _get_nc(npow)
    results = _get_runner(nc)(in_maps)
    return assemble_output(results)
